# revision 27
# baseline (speedup 1.0000x reference)
"""GroupQueryAttention (B=1, S=2048, H=4096, 32 Q heads, 8 KV groups) on 8
Trainium2 NeuronCores, tensor-parallel over heads.

Sharding: core c owns Q heads 4c..4c+3 and KV group c. The reference's head
merge is `swapaxes(-1,-2).reshape`, which makes output row r = 64*h + d//2 and
column 2048*(d%2) + s -- i.e. each output row depends on exactly one head, so
the o-projection is row-parallel across cores with NO cross-core reduction.
Core c produces output rows [256c, 256c+256).

Device math per core (all matmuls bf16 with fp32 PSUM accumulation):
  Q^T[d,s]   = wq_c^T @ X^T        (1/sqrt(d) and bq folded into wq on host)
  K^T[d,s]   = wk_c^T @ X^T
  V[s,d]     = X @ wv_c            (lhsT = X^T k-tiles)
  S^T[sk,sq] = lhsT=K^T[:,sk], rhs=Q^T[:,sq]
  E = exp(S^T)                     (no max subtraction: |scores| <~ 10)
  N[sq,0:128] = sum_sk E^T V ; N[sq,128] = denom (ones column appended to V)
  O = N[:, :128] * (1/N[:,128])    stored interleaved by d-parity for o-proj
  out rows = Y_c @ wo + bo         (Y_c^T k-tiles are strided views of O)
Biases enter via one extra contraction tile (row 4096 of padded operands = bias,
paired with an all-ones row-0 operand on the other side).

Schedule (v2), engine-overlap oriented:
  front:  pass c0* = k-outer {V + K c0..c3} with the bias k-tile first (xt
          streamed in, PE covers the DMA); pass1 = k-outer {Q c0, c1} (one
          wqt stream feeds both chunks, banks staggered so the c0* PSUM
          copies drain); pass2 = k-outer {Q c2} (+ Exp-table preload).
          Q chunk3 is deferred into the attention phase.
  attention: flat 256-step pipeline over (chunk, head, sk): scores are
          emitted 2 steps ahead of PV so ScalarE's exp latency is hidden;
          each step additionally emits "fill" PE work:
            chunk0 steps -> Q chunk3 matmuls (head-pair passes, 2 PSUM banks)
            chunk c>=1 steps -> o-projection matmuls for chunk c-1
          o-projection accumulates 8 k-tiles per (jc, mt) group in one PSUM
          bank, then flush-adds into an SBUF fp32 accumulator (DVE), with wo
          streamed through a prefetch ring.
  tail:   o-projection for chunk3 + bias tile in [128,2,512] wo tiles with
          2-group DMA lookahead; output DMAs issue per half-region.
PSUM rule respected throughout: matmul start=True clears the whole 2KB bank,
so each bank hosts exactly one accumulation group (start on first write into
the bank, stop on the last; packed sub-tiles accumulate onto the cleared
zeros).  fp8/DoubleRow was evaluated and rejected: e4m3 quantization of Q
alone yields ~5% output error (score noise passes through softmax 1:1).
"""

import math
from contextlib import ExitStack

import ml_dtypes
import numpy as np

P = 128
S = 2048
HID = 4096
KPAD = HID + P          # 4224: one extra k-tile for the bias trick
KT = KPAD // P          # 33 contraction tiles
CH = 4                  # 512-wide seq chunks
SKT = 16                # 128-row sk tiles
NCORES = 8
HPC = 4                 # heads per core
NJC = 8                 # 512-wide column slices of the o-projection output
BF16 = ml_dtypes.bfloat16

_CACHE = {}

PROFILE = False         # set True (e.g. from test.py) to capture an NTFF trace


def _build_nc():
    import concourse.tile as tile
    from concourse import bacc, mybir

    f32 = mybir.dt.float32
    bf16 = mybir.dt.bfloat16
    Exp = mybir.ActivationFunctionType.Exp
    Mult = mybir.AluOpType.mult
    Add = mybir.AluOpType.add

    nc = bacc.Bacc("TRN2", target_bir_lowering=False, debug=False)

    xt_d = nc.dram_tensor("xt", [KPAD, S], bf16, kind="ExternalInput").ap()
    wq_d = nc.dram_tensor("wq", [KPAD, 512], bf16, kind="ExternalInput").ap()
    wk_d = nc.dram_tensor("wk", [KPAD, P], bf16, kind="ExternalInput").ap()
    wv_d = nc.dram_tensor("wv", [KPAD, P], bf16, kind="ExternalInput").ap()
    # wo pre-tiled on host: row block (kt*NJC + jc)*P .. +P holds the
    # [128, 512] tile wo_pad[kt*128:(kt+1)*128, jc*512:(jc+1)*512].
    wo_d = nc.dram_tensor("wo", [KT * NJC * P, 512], bf16, kind="ExternalInput").ap()
    ones_d = nc.dram_tensor("ones", [P, 512], bf16, kind="ExternalInput").ap()
    out_d = nc.dram_tensor("out", [2 * P, HID], f32, kind="ExternalOutput").ap()

    with tile.TileContext(nc) as tc, ExitStack() as ctx:
        pers = ctx.enter_context(tc.tile_pool(name="pers", bufs=1))
        attn = ctx.enter_context(tc.tile_pool(name="attn", bufs=1))

        qt = [pers.tile([P, S], bf16, name=f"qt{h}", tag=f"qt{h}") for h in range(HPC)]
        ktr = pers.tile([P, S], bf16, name="ktr", tag="ktr")
        # V with a ones column at 128 so the PV matmul also emits the denom
        v1 = pers.tile([P, SKT, 132], bf16, name="v1", tag="v1")
        # O interleaved: [s_local, pair, s_tile, d%2, head_in_pair, d//2]
        opair = pers.tile([P, 2, SKT, 2, 2, 64], bf16, name="opair", tag="opair")
        ones_sb = pers.tile([P, 512], bf16, name="ones_sb", tag="ones_sb")
        wv = pers.tile([P, KT, P], bf16, name="wv", tag="wv")

        qkv_ctx = ctx.enter_context(ExitStack())
        qkv = qkv_ctx.enter_context(tc.tile_pool(name="qkv", bufs=1))

        xt = [
            qkv.tile([P, S], bf16, name=f"xt{k}", tag=f"xt{k}")
            for k in range(KT - 1)
        ]

        def xrhs(k, c):
            # rhs [128,512] for contraction tile k, seq chunk c; tile 32 is
            # the bias tile: row 4096 of X^T_pad is all-ones.
            if k < KT - 1:
                return xt[k][:, c * 512:(c + 1) * 512]
            return ones_sb[:]

        def new_wqt():
            # bufs must cover pass1's 4-step bank stagger: a wqt tile is read
            # again 4 allocations later, so 8 buffers keep WAR clean
            return qkv.tile([P, 512], bf16, name="wqt", tag="wqt", bufs=6)

        wv_r = wv_d.rearrange("(k p) d -> p k d", p=P)

        with tc.tile_pool(name="psF", bufs=1, space="PSUM") as psF:
            # ---- pass c0*: k-outer {V + K c0..c3}, xt streamed.  Only xt
            # and the small wkt/wv tiles move, so the PE (1.7us/step) covers
            # the 1.5us/step of DMA with margin. ----
            kp = [psF.tile([P, 512], f32, name=f"kp{c}", tag=f"t{c}")
                  for c in range(CH)]
            vp = [psF.tile([P, 512], f32, name=f"vp{j}", tag=f"t{4 + j}")
                  for j in range(4)]

            def new_wkt():
                return qkv.tile([P, P], bf16, name="wkt", tag="wkt", bufs=6)

            # k-order: the bias tile (ones_sb + tiny weights) goes FIRST so
            # the PE has a full step of work while xt[0]/xt[1] stream in.
            korder = [KT - 1] + list(range(KT - 1))
            # prologue DMAs; xt0 split into chunk pieces so each of the first
            # K matmuls starts as soon as its own 512 columns land
            wkts = {KT - 1: new_wkt()}
            nc.sync.dma_start(wkts[KT - 1][:], wk_d[(KT - 1) * P:KT * P, :])
            nc.sync.dma_start(ones_sb[:], ones_d[:])
            nc.sync.dma_start(wv[:, KT - 1:KT, :], wv_r[:, KT - 1:KT, :])
            wkts[0] = new_wkt()
            nc.sync.dma_start(wkts[0][:], wk_d[0:P, :])
            nc.sync.dma_start(xt[0][:], xt_d[0:P, :])
            nc.sync.dma_start(wv[:, 0:4, :], wv_r[:, 0:4, :])
            wkts[1] = new_wkt()
            nc.sync.dma_start(wkts[1][:], wk_d[P:2 * P, :])
            nc.sync.dma_start(xt[1][:], xt_d[P:2 * P, :])

            wv_pieces = [(a, min(a + 2, KT - 1)) for a in range(4, KT - 1, 2)]
            for step, k in enumerate(korder):
                if step + 2 < len(korder):
                    k2 = korder[step + 2]
                    if k2 < KT - 1 and k2 >= 2:
                        nc.sync.dma_start(
                            xt[k2][:], xt_d[k2 * P:(k2 + 1) * P, :])
                    if k2 >= 2:
                        w = new_wkt()
                        nc.sync.dma_start(w[:], wk_d[k2 * P:(k2 + 1) * P, :])
                        wkts[k2] = w
                if step - 1 < len(wv_pieces) and step >= 1:
                    a, b = wv_pieces[step - 1]
                    nc.sync.dma_start(wv[:, a:b, :], wv_r[:, a:b, :])
                for c in range(CH):
                    nc.tensor.matmul(
                        kp[c][:], wkts[k][:], xrhs(k, c),
                        start=(step == 0), stop=(step == len(korder) - 1),
                    )
                for sk in range(SKT):
                    if k < KT - 1:
                        lhs = xt[k][:, sk * P:(sk + 1) * P]
                    else:
                        lhs = ones_sb[:, :P]
                    # one accumulation group per PSUM bank: start=True clears
                    # the whole bank, so only the first write into the bank
                    # starts and only the last stops
                    nc.tensor.matmul(
                        vp[sk // 4][:, (sk % 4) * P:(sk % 4 + 1) * P],
                        lhs, wv[:, k, :],
                        start=(step == 0 and sk % 4 == 0),
                        stop=(step == len(korder) - 1 and sk % 4 == 3),
                    )
            wkts = None
            # free the PSUM banks fast: kp copies lead (pass1 uses t0 first),
            # split between DVE and ScalarE so they drain in parallel
            for c in range(CH):
                dst = ktr[:, c * 512:(c + 1) * 512]
                if c % 2 == 0:
                    nc.vector.tensor_copy(dst, kp[c][:])
                else:
                    nc.scalar.copy(dst, kp[c][:])
            for sk in range(SKT):
                dst = v1[:, sk, :P]
                s_ = vp[sk // 4][:, (sk % 4) * P:(sk % 4 + 1) * P]
                if sk % 2 == 0:
                    nc.vector.tensor_copy(dst, s_)
                else:
                    nc.scalar.copy(dst, s_)

            # ---- pass1: k-outer {Q c0, c1}; one wqt stream feeds both
            # chunks; banks t4-7 start 4 k-steps late so c0*'s vp copies
            # have drained their banks ----
            STG = 4
            qp2 = [psF.tile([P, 512], f32, name=f"qp2_{i}", tag=f"t{i}")
                   for i in range(8)]
            wqts = [None] * KT
            for k in range(KT + STG):
                if k < KT:
                    wqt = new_wqt()
                    nc.sync.dma_start(wqt[:], wq_d[k * P:(k + 1) * P, :])
                    wqts[k] = wqt
                for i in range(8):
                    kk = k if i < 4 else k - STG
                    if not (0 <= kk < KT):
                        continue
                    h, c = i % HPC, i // HPC
                    nc.tensor.matmul(
                        qp2[i][:], wqts[kk][:, h * P:(h + 1) * P], xrhs(kk, c),
                        start=(kk == 0), stop=(kk == KT - 1),
                    )
                    if kk == KT - 1:
                        # copy out as soon as this bank's accumulation stops
                        dst = qt[h][:, c * 512:(c + 1) * 512]
                        if i % 2 == 0:
                            nc.vector.tensor_copy(dst, qp2[i][:])
                        else:
                            nc.scalar.copy(dst, qp2[i][:])
            wqts = None

            # ---- pass2: k-outer {Q c2}, banks staggered ----
            qp3 = [psF.tile([P, 512], f32, name=f"qp3_{h}", tag=f"t{h}")
                   for h in range(HPC)]
            wqts = [None] * KT
            for k in range(KT + HPC - 1):
                if k < KT:
                    wqt = new_wqt()
                    nc.sync.dma_start(wqt[:], wq_d[k * P:(k + 1) * P, :])
                    wqts[k] = wqt
                if k == 2:
                    # preload the Exp activation table while the PE crunches:
                    # the first real exp then avoids a 1.3us table swap right
                    # at the head of the attention pipeline
                    scratch = attn.tile([P, 1], bf16, name="scratch",
                                        tag="scratch")
                    nc.scalar.activation(scratch[:], ones_sb[:, 0:1], Exp)
                for h in range(HPC):
                    # stop order t0,t2,t1,t3: frees the banks the attention
                    # nps tiles land on as early as possible
                    kk = k - [0, 2, 1, 3][h]
                    if not (0 <= kk < KT):
                        continue
                    nc.tensor.matmul(
                        qp3[h][:], wqts[kk][:, h * P:(h + 1) * P], xrhs(kk, 2),
                        start=(kk == 0), stop=(kk == KT - 1),
                    )
                    if kk == KT - 1:
                        # DVE only: ScalarE here would swap the activation
                        # table back to Copy right after the Exp preload
                        nc.vector.tensor_copy(
                            qt[h][:, 2 * 512:3 * 512], qp3[h][:])
            wqts = None
            # ones column for the PV denominator; first read is attention
            # step 0, so emitting it here keeps it off the startup DMA chain
            nc.vector.memset(v1[:, :, 128:129], 1.0)

        # ================= attention + interleaved fills =================
        with ExitStack() as act_ctx:
            psA = act_ctx.enter_context(
                tc.tile_pool(name="psA", bufs=1, space="PSUM")
            )
            psQ_ctx = ExitStack()
            psQ = psQ_ctx.enter_context(
                tc.tile_pool(name="psQ", bufs=1, space="PSUM")
            )
            state = {}

            NSTEP = CH * HPC * SKT  # 256

            def step_unit(i):
                u, sk = divmod(i, SKT)
                c, h = divmod(u, HPC)
                return h, c, sk

            def emit_S(i):
                h, c, sk = step_unit(i)
                sp = psA.tile([P, 512], f32, name="sp", tag="sp", bufs=2)
                nc.tensor.matmul(
                    sp[:], ktr[:, sk * P:(sk + 1) * P],
                    qt[h][:, c * 512:(c + 1) * 512],
                    start=True, stop=True,
                )
                return sp

            def qc3_fill():
                # Q chunk3, two heads per pass: 2 PSUM banks
                for hp in range(2):
                    qfs = [
                        psQ.tile([P, 512], f32, name=f"qf{t}", tag="qf", bufs=2)
                        for t in range(2)
                    ]
                    for k in range(KT):
                        w = new_wqt()
                        nc.sync.dma_start(w[:], wq_d[k * P:(k + 1) * P, :])
                        for t in range(2):
                            h = 2 * hp + t
                            nc.tensor.matmul(
                                qfs[t][:], w[:, h * P:(h + 1) * P], xrhs(k, 3),
                                start=(k == 0), stop=(k == KT - 1),
                            )
                            yield
                    for t in range(2):
                        nc.vector.tensor_copy(
                            qt[2 * hp + t][:, 3 * 512:4 * 512], qfs[t][:]
                        )

            def oproj_group(c, kts, jc, mt, mov):
                # one (jc, mt) accumulation group: 8-9 matmuls into a PSUM
                # bank, then flush-add into the SBUF accumulator
                psO = state["psO"]
                out_acc = state["out_acc"]
                opb = psO.tile([P, 512], f32, name="opb", tag="opb", bufs=2)
                for i, kt in enumerate(kts):
                    st, par = kt % SKT, kt // SKT
                    if kt < KT - 1:
                        lhs = opair[:, mt, st, par, :, :]
                    else:
                        lhs = ones_sb[:, :P]
                    nc.tensor.matmul(
                        opb[:], lhs, mov(i),
                        start=(i == 0), stop=(i == len(kts) - 1),
                    )
                    yield
                acc = out_acc[mt][:, jc * 512:(jc + 1) * 512]
                if c == 0:
                    nc.vector.tensor_copy(acc, opb[:])
                elif c < CH - 1:
                    nc.vector.scalar_tensor_tensor(
                        acc, opb[:], 1.0, acc, Mult, Add,
                    )
                else:
                    # final chunk: flush and ship in halves so the last
                    # group's flush->DMA chain is half as long
                    for hh in range(2):
                        a_ = acc[:, hh * 256:(hh + 1) * 256]
                        nc.vector.scalar_tensor_tensor(
                            a_, opb[:, hh * 256:(hh + 1) * 256], 1.0, a_,
                            Mult, Add,
                        )
                        nc.scalar.dma_start(
                            out_d[mt * P:(mt + 1) * P,
                                  jc * 512 + hh * 256:jc * 512 + (hh + 1) * 256],
                            a_,
                        )

            def oproj_fill(c):
                # o-projection matmuls fed by chunk c's opair slices,
                # wo streamed through a prefetch ring
                kts = [4 * c + i for i in range(4)] + \
                      [16 + 4 * c + i for i in range(4)]

                def dma_jc(jc):
                    ws = []
                    for kt in kts:
                        w = attn.tile([P, 512], bf16, name="wot",
                                      tag="wot", bufs=18)
                        r = (kt * NJC + jc) * P
                        nc.sync.dma_start(w[:], wo_d[r:r + P, :])
                        ws.append(w)
                    return ws

                wots = dma_jc(0)
                yield  # pre-pump marker: jc=0 DMAs are in flight
                for jc in range(NJC):
                    cur = wots
                    wots = dma_jc(jc + 1) if jc + 1 < NJC else None
                    for mt in range(2):
                        yield from oproj_group(c, kts, jc, mt,
                                               lambda i: cur[i][:])

            def oproj_tail():
                # chunk3 + bias: nothing overlaps this, so it is DMA-rate
                # sensitive -- use [128, 2, 512] wo tiles (one DMA per
                # jc-pair) from the roomy late pool, 2 groups of lookahead
                c = CH - 1
                late = state["late"]
                kts = [4 * c + i for i in range(4)] + \
                      [16 + 4 * c + i for i in range(4)] + [KT - 1]

                def dma_jcp(u):
                    ws = []
                    for kt in kts:
                        w = late.tile([P, 2, 512], bf16, name="wotw",
                                      tag="wotw", bufs=27)
                        r = (kt * NJC + 2 * u) * P
                        nc.sync.dma_start(
                            w[:], wo_d[r:r + 2 * P, :].rearrange(
                                "(b p) d -> p b d", p=P)
                        )
                        ws.append(w)
                    return ws

                pend = [dma_jcp(0), dma_jcp(1)]
                for u in range(NJC // 2):
                    cur = pend.pop(0)
                    if u + 2 < NJC // 2:
                        pend.append(dma_jcp(u + 2))
                    for half in range(2):
                        for mt in range(2):
                            yield from oproj_group(
                                c, kts, 2 * u + half, mt,
                                lambda i, _c=cur, _h=half: _c[i][:, _h, :],
                            )

            def drain(g):
                if g is not None:
                    for _ in g:
                        pass

            FILLN = [2, 2, 2, 2]   # fill draws per step while in chunk c
            cur_fill = qc3_fill()

            def new_nps():
                a = psA.tile([P, 512], f32, name="npsA", tag="npsA", bufs=2)
                b = psA.tile([P, 512], f32, name="npsB", tag="npsB", bufs=2)
                return a, b

            npsA, npsB = new_nps()
            sps = {0: emit_S(0), 1: emit_S(1)}
            for i in range(NSTEP):
                h, c, sk = step_unit(i)
                e = attn.tile([P, 512], bf16, name="es", tag="es", bufs=3)
                nc.scalar.activation(e[:], sps.pop(i)[:], Exp)
                if i + 2 < NSTEP:
                    sps[i + 2] = emit_S(i + 2)
                pair, j = divmod(h, 2)
                for q in range(4):
                    bank, off = (npsA, 129 * q) if q < 3 else (npsB, 0)
                    # npsA carries three packed 129-wide groups in one bank:
                    # start only on the first (q==0) write, stop on the last
                    nc.tensor.matmul(
                        bank[:, off:off + 129], e[:, q * P:(q + 1) * P],
                        v1[:, sk, :129],
                        start=(sk == 0 and q in (0, 3)),
                        stop=(sk == SKT - 1 and q in (2, 3)),
                    )
                if cur_fill is not None:
                    for _ in range(FILLN[c]):
                        try:
                            next(cur_fill)
                        except StopIteration:
                            cur_fill = None
                            break
                if sk == SKT - 1:
                    # unit boundary: normalize + scatter into opair
                    for q in range(4):
                        bank, off = (npsA, 129 * q) if q < 3 else (npsB, 0)
                        st = c * 4 + q
                        rc = attn.tile([P, 1], f32, name="rc", tag="rc", bufs=4)
                        nc.vector.reciprocal(rc[:], bank[:, off + 128:off + 129])
                        for par in range(2):
                            nc.vector.tensor_scalar(
                                opair[:, pair, st, par, j, :],
                                bank[:, off + par:off + 128:2], rc[:],
                                None, Mult,
                            )
                    if h == HPC - 1:
                        # chunk boundary
                        drain(cur_fill)
                        if c == 0:
                            psQ_ctx.close()
                            qkv_ctx.close()
                            state["psO"] = act_ctx.enter_context(
                                tc.tile_pool(name="psO", bufs=1, space="PSUM")
                            )
                            late = act_ctx.enter_context(
                                tc.tile_pool(name="late", bufs=1)
                            )
                            state["late"] = late
                            state["out_acc"] = [
                                late.tile([P, HID], f32, name=f"oacc{mt}",
                                          tag=f"oacc{mt}")
                                for mt in range(2)
                            ]
                        if c < CH - 1:
                            cur_fill = oproj_fill(c)
                            next(cur_fill)  # pre-pump jc0 DMAs
                    if i + 1 < NSTEP:
                        npsA, npsB = new_nps()

            # tail: o-projection for chunk3 (+ bias); output DMAs inline
            drain(oproj_tail())

    nc.compile()
    return nc


def _get_nc():
    if "nc" not in _CACHE:
        _CACHE["nc"] = _build_nc()
    return _CACHE["nc"]


def kernel(hidden_state, wq, bq, wk, bk, wv, bv, wo, bo):
    from concourse import bass_utils

    nc = _get_nc()

    X = np.asarray(hidden_state, np.float32).reshape(S, HID)
    scale = 1.0 / math.sqrt(P)

    xt_pad = np.zeros((KPAD, S), np.float32)
    xt_pad[:HID] = X.T
    xt_pad[HID] = 1.0
    xt_bf = xt_pad.astype(BF16)

    wo_pad = np.zeros((KPAD, HID), np.float32)
    wo_pad[:HID] = np.asarray(wo, np.float32)
    wo_pad[HID] = np.asarray(bo, np.float32)
    # pre-tile: [KT, 128, NJC, 512] -> [KT, NJC, 128, 512] -> flat rows
    wo_bf = (
        wo_pad.reshape(KT, P, NJC, 512)
        .transpose(0, 2, 1, 3)
        .reshape(KT * NJC * P, 512)
        .astype(BF16)
    )

    ones_np = np.zeros((P, 512), np.float32)
    ones_np[0] = 1.0
    ones_bf = ones_np.astype(BF16)

    wq = np.asarray(wq, np.float32)
    bq = np.asarray(bq, np.float32)
    wk = np.asarray(wk, np.float32)
    bk = np.asarray(bk, np.float32)
    wv = np.asarray(wv, np.float32)
    bv = np.asarray(bv, np.float32)

    in_maps = []
    for c in range(NCORES):
        wq_pad = np.zeros((KPAD, 512), np.float32)
        wq_pad[:HID] = wq[:, c * 512:(c + 1) * 512] * scale
        wq_pad[HID] = bq[c * 512:(c + 1) * 512] * scale
        wk_pad = np.zeros((KPAD, P), np.float32)
        wk_pad[:HID] = wk[:, c * P:(c + 1) * P]
        wk_pad[HID] = bk[c * P:(c + 1) * P]
        wv_pad = np.zeros((KPAD, P), np.float32)
        wv_pad[:HID] = wv[:, c * P:(c + 1) * P]
        wv_pad[HID] = bv[c * P:(c + 1) * P]
        in_maps.append({
            "xt": xt_bf,
            "wq": wq_pad.astype(BF16),
            "wk": wk_pad.astype(BF16),
            "wv": wv_pad.astype(BF16),
            "wo": wo_bf,
            "ones": ones_bf,
        })

    try:
        res = bass_utils.run_bass_kernel_spmd(
            nc, in_maps, core_ids=list(range(NCORES)), trace=PROFILE,
        )
    except ModuleNotFoundError:
        # NTFF profile hook unavailable in this environment
        res = bass_utils.run_bass_kernel_spmd(
            nc, in_maps, core_ids=list(range(NCORES)), trace=False,
        )
    _CACHE["last_results"] = res

    out = np.empty((1, S, HID), np.float32)
    for c in range(NCORES):
        out[0, c * 256:(c + 1) * 256, :] = res.results[c]["out"]
    return out
